# revision 5
# baseline (speedup 1.0000x reference)
"""Chamfer distance kernel for Trainium2 (8 NeuronCores).

Strategy
--------
dist[b,i,j] = ||pred[b,j] - gt[b,i]||.  The chamfer value needs
min_j dist (per gt row) and min_i dist (per pred col).  Since sqrt is
monotone, mins are taken over *squared* distances and the sqrt is applied
to the reduced vectors on the host.

The squared distances are produced directly in PSUM by one augmented
matmul: neg_sq[i,j] = 2*gt[i].pred[j] - |gt[i]|^2 - |pred[j]|^2 (negated
so all reductions become max, which every engine supports).  fp32 matmul
on TRN2 runs at 4 cycles/row, so instead the fp32 operands are split into
bf16 triples (h+m+l recovers 24 mantissa bits) and the products expanded:
g.P = gh.Ph + gh.Pm + gm.Ph + gh.Pl + gl.Ph + gm.Pm  (+ O(2^-24) terms)
With the norm rows this gives a K=24 bf16 matmul (1 cycle/row) whose
result matches the fp32 expansion to ~1e-6.

Sharding: gt rows are split across the 8 cores (1024 rows/core, both
batches).  Each core computes its [2048 x 16384] slab of the (negated)
distance matrix in [128 x 2048] PSUM strips:
  - row-max (per gt row) via DVE tensor_scalar with max-accum fused into
    the PSUM->SBUF(bf16) eviction, or ACT copy + 4x bf16 DVE max,
  - col-max accumulated across row tiles via bf16 tensor_tensor max
    (DVE 2x mode, some groups on GPSIMD).
Outputs per core: rowmax [128,16] fp32 and colmax [128,16384] bf16.
The host concatenates row mins, folds the 128 colacc partitions and the
8 cores with np.max, applies sqrt and the means.
"""

import os
import sys
import types
import numpy as np
import ml_dtypes

# ---------------------------------------------------------------------------
# problem constants (hardcoded per spec: pred/gt [2, 8192, 3] fp32)
B = 2
N = 8192
NCORES = 8
GPC = N // NCORES          # gt rows per core per batch = 1024
RT = GPC // 128            # row tiles per batch per core = 8
CB = 4                     # col blocks per batch (each 2048 preds)
CBW = N // CB              # col block width = 2048
NSTRIP = B * CB * RT       # 64 strips per core
K = 24                     # contraction rows of the augmented matmul

# engine assignment knobs (see module docstring)
DVE_EVICT_T = {0: True, 4: None}   # t=0 always DVE-evict; t=4 on even groups
GP_GROUPS = ()                     # (b*CB+cb) groups whose colacc runs on GPSIMD
                                   # (walrus rejects Pool TensorTensor on fp16)

_BF16 = ml_dtypes.bfloat16


def _ensure_concourse():
    for p in ("/root/.axon_site", "/root/.axon_site/_ro/trn_rl_repo",
              "/root/.axon_site/_ro/pypackages", "/opt/trn_rl_repo"):
        if os.path.isdir(p) and p not in sys.path:
            sys.path.append(p)


def _split3(x64):
    """Split a float64 array into three bf16 components summing to ~24 bits."""
    h = x64.astype(_BF16)
    r = x64 - h.astype(np.float64)
    m = r.astype(_BF16)
    r2 = r - m.astype(np.float64)
    l = r2.astype(_BF16)
    return h, m, l


def _build_aug(pred, gt):
    """Build aug_pred [K, B*N] and aug_gt [K, B*N] bf16 host arrays.

    Row pairing k: lhsT[k] (gt side) x rhs[k] (pred side):
      0-2   gh . Ph      3-5   gh . Pm      6-8   gm . Ph
      9-11  gh . Pl     12-14  gl . Ph     15-17  gm . Pm
      18-20 gsq{h,m,l} . (-1)              21-23  1 . (-psq{h,m,l})
    where P = 2*pred.
    """
    g64 = gt.astype(np.float64).reshape(B * N, 3)
    P64 = (2.0 * pred.astype(np.float64)).reshape(B * N, 3)
    gsq = (gt.astype(np.float32) ** 2).sum(-1, dtype=np.float32).astype(np.float64).reshape(B * N)
    psq = (pred.astype(np.float32) ** 2).sum(-1, dtype=np.float32).astype(np.float64).reshape(B * N)

    gh, gm, gl = _split3(g64)
    Ph, Pm, Pl = _split3(P64)
    gsqh, gsqm, gsql = _split3(gsq)
    psqh, psqm, psql = _split3(psq)

    one = np.ones(B * N, _BF16)
    neg1 = np.full(B * N, -1.0, _BF16)

    def rows3(a):  # [B*N, 3] -> 3 rows
        return [a[:, 0], a[:, 1], a[:, 2]]

    aug_gt = np.stack(
        rows3(gh) + rows3(gh) + rows3(gm) + rows3(gh) + rows3(gl) + rows3(gm)
        + [gsqh, gsqm, gsql, one, one, one], axis=0)
    aug_pred = np.stack(
        rows3(Ph) + rows3(Pm) + rows3(Ph) + rows3(Pl) + rows3(Ph) + rows3(Pm)
        + [neg1, neg1, neg1, -psqh, -psqm, -psql], axis=0)
    assert aug_gt.shape == (K, B * N) and aug_pred.shape == (K, B * N)
    return aug_gt, aug_pred


def _use_dve_evict(group, t):
    v = DVE_EVICT_T.get(t)
    if v is None:
        return group % 2 == 0
    return bool(v)


def build_nc():
    """Trace + compile the single-program SPMD kernel. Returns the Bacc."""
    _ensure_concourse()
    from contextlib import ExitStack
    import concourse.tile as tile
    from concourse import bacc, mybir

    f32 = mybir.dt.float32
    bf16 = mybir.dt.bfloat16
    f16 = mybir.dt.float16
    MAX = mybir.AluOpType.max
    ADD = mybir.AluOpType.add

    nc = bacc.Bacc("TRN2", target_bir_lowering=False, debug=False,
                   enable_asserts=False, num_devices=NCORES)
    ag_d = nc.dram_tensor("aug_gt", [K, B * GPC], bf16, kind="ExternalInput").ap()
    ap_d = nc.dram_tensor("aug_pred", [K, B * N], bf16, kind="ExternalInput").ap()
    rmax_d = nc.dram_tensor("rowmax_out", [128, B * RT], f32, kind="ExternalOutput").ap()
    cmax_d = nc.dram_tensor("colmax_out", [128, B * N], f16, kind="ExternalOutput").ap()

    with tile.TileContext(nc) as tc, ExitStack() as ctx:
        const_pool = ctx.enter_context(tc.tile_pool(name="const", bufs=1))
        psum_pool = ctx.enter_context(tc.tile_pool(name="ps", bufs=2, space="PSUM"))
        bpool = ctx.enter_context(tc.tile_pool(name="bs", bufs=4))
        gpool = ctx.enter_context(tc.tile_pool(name="bsgp", bufs=10))
        dpool = ctx.enter_context(tc.tile_pool(name="dm", bufs=2))

        ag = const_pool.tile([K, B * GPC], bf16)
        nc.sync.dma_start(ag[:], ag_d[:])
        apt = const_pool.tile([K, B * N], bf16)
        nc.sync.dma_start(apt[:], ap_d[:])
        colacc = const_pool.tile([128, B * N], f16)
        rowparts = const_pool.tile([128, NSTRIP], f32)

        for b in range(B):
            for cb in range(CB):
                group = b * CB + cb
                on_gp = group in GP_GROUPS
                ccol = b * N + cb * CBW
                cc = colacc[:, ccol:ccol + CBW]
                for t in range(RT):
                    s = group * RT + t
                    psum = psum_pool.tile([128, CBW], f32, tag="ps")
                    wcol = (b * RT + t) * 128
                    for n in range(4):
                        nc.tensor.matmul(
                            psum[:, n * 512:(n + 1) * 512],
                            lhsT=ag[:, wcol:wcol + 128],
                            rhs=apt[:, ccol + n * 512: ccol + (n + 1) * 512],
                            start=True, stop=True)
                    pool = gpool if on_gp else bpool
                    bstrip = pool.tile([128, CBW], f16, tag="gp" if on_gp else "bs")
                    if _use_dve_evict(group, t):
                        nc.vector.tensor_scalar(
                            out=bstrip[:], in0=psum[:], scalar1=0.0, scalar2=None,
                            op0=ADD, op1=MAX, accum_out=rowparts[:, s:s + 1])
                    else:
                        nc.scalar.activation(bstrip[:], psum[:],
                                             mybir.ActivationFunctionType.Copy)
                        dummy = dpool.tile([128, CBW], f16, tag="dm")
                        nc.vector.tensor_scalar(
                            out=dummy[:], in0=bstrip[:], scalar1=0.0, scalar2=None,
                            op0=ADD, op1=MAX, accum_out=rowparts[:, s:s + 1])
                    if t == 0:
                        nc.vector.tensor_copy(cc[:], bstrip[:])
                    else:
                        eng = nc.gpsimd if on_gp else nc.vector
                        eng.tensor_tensor(out=cc[:], in0=cc[:], in1=bstrip[:], op=MAX)
                nc.sync.dma_start(cmax_d[:, ccol:ccol + CBW], cc[:])

        rfin = const_pool.tile([128, B * RT], f32)
        view = rowparts[:].rearrange("p (b cb t) -> p b t cb", b=B, cb=CB, t=RT)
        nc.vector.tensor_reduce(out=rfin[:], in_=view,
                                axis=mybir.AxisListType.X, op=MAX)
        nc.sync.dma_start(rmax_d[:], rfin[:])

    nc.compile()
    return nc


_NC_CACHE = None


def _get_nc():
    global _NC_CACHE
    if _NC_CACHE is None:
        _NC_CACHE = build_nc()
    return _NC_CACHE


def make_in_maps(pred, gt):
    """Per-core input dicts. Core c gets gt rows [c*GPC, (c+1)*GPC) of each
    batch (aug_gt columns laid out b-major: (b*RT + t)*128 + p)."""
    aug_gt, aug_pred = _build_aug(pred, gt)
    ag_bn = aug_gt.reshape(K, B, N)
    in_maps = []
    for c in range(NCORES):
        ag_c = ag_bn[:, :, c * GPC:(c + 1) * GPC].reshape(K, B * GPC)
        in_maps.append({"aug_gt": np.ascontiguousarray(ag_c),
                        "aug_pred": np.ascontiguousarray(aug_pred)})
    return in_maps


def finalize(results):
    """Host finale: negated maxes -> mins -> sqrt -> means."""
    # rowmax_out: [128, B*RT], col = b*RT + t, partition p -> gt row c*GPC + t*128 + p
    dist1_sq = np.empty((B, N), np.float64)
    for c in range(NCORES):
        r = np.asarray(results[c]["rowmax_out"], np.float64)  # [128, B*RT]
        r = r.reshape(128, B, RT).transpose(1, 2, 0).reshape(B, GPC)
        dist1_sq[:, c * GPC:(c + 1) * GPC] = -r
    # colmax_out: [128, B*N] fp16 per core; fold cores and partitions
    call = np.stack([np.asarray(results[c]["colmax_out"]).astype(np.float32)
                     for c in range(NCORES)], axis=0)  # [8, 128, B*N]
    dist2_sq = -(call.max(axis=(0, 1)).astype(np.float64).reshape(B, N))

    dist1 = np.sqrt(np.maximum(dist1_sq, 0.0))
    dist2 = np.sqrt(np.maximum(dist2_sq, 0.0))
    chamfer = (dist1.mean(axis=1) + dist2.mean(axis=1)).mean()
    return np.float32(chamfer)


def kernel(pred, gt):
    _ensure_concourse()
    pred = np.asarray(pred, dtype=np.float32)
    gt = np.asarray(gt, dtype=np.float32)
    assert pred.shape == (B, N, 3) and gt.shape == (B, N, 3)

    in_maps = make_in_maps(pred, gt)
    nc = _get_nc()
    from concourse import bass_utils
    res = bass_utils.run_bass_kernel_spmd(nc, in_maps, core_ids=list(range(NCORES)))
    return finalize(res.results)


# revision 9
# speedup vs baseline: 1.3266x; 1.3266x over previous
"""Chamfer distance kernel for Trainium2 (8 NeuronCores).

Strategy
--------
dist[b,i,j] = ||pred[b,j] - gt[b,i]||.  The chamfer value needs
min_j dist (per gt row) and min_i dist (per pred col).  Since sqrt is
monotone, mins are taken over *squared* distances and the sqrt is applied
to the reduced vectors on the host.

The squared distances are produced directly in PSUM by one augmented
matmul: neg_sq[i,j] = 2*gt[i].pred[j] - |gt[i]|^2 - |pred[j]|^2 (negated
so all reductions become max, which every engine supports).  fp32 matmul
on TRN2 runs at 4 cycles/row, so instead the fp32 operands are split into
bf16 triples (h+m+l recovers 24 mantissa bits) and the products expanded:
g.P = gh.Ph + gh.Pm + gm.Ph + gh.Pl + gl.Ph + gm.Pm  (+ O(2^-24) terms)
With the norm rows this gives a K=24 bf16 matmul (1 cycle/row) whose
result matches the fp32 expansion to ~1e-6.

Sharding: gt rows are split across the 8 cores (1024 rows/core, both
batches).  Each core computes its [2048 x 16384] slab of the (negated)
distance matrix in [128 x 2048] PSUM strips:
  - row-max (per gt row) via DVE tensor_scalar with max-accum fused into
    the PSUM->SBUF(bf16) eviction, or ACT copy + 4x bf16 DVE max,
  - col-max accumulated across row tiles via bf16 tensor_tensor max
    (DVE 2x mode, some groups on GPSIMD).
Outputs per core: rowmax [128,16] fp32 and colmax [128,16384] bf16.
The host concatenates row mins, folds the 128 colacc partitions and the
8 cores with np.max, applies sqrt and the means.
"""

import os
import sys
import types
import numpy as np
import ml_dtypes

# ---------------------------------------------------------------------------
# problem constants (hardcoded per spec: pred/gt [2, 8192, 3] fp32)
B = 2
N = 8192
NCORES = 8
GPC = N // NCORES          # gt rows per core per batch = 1024
RT = GPC // 128            # row tiles per batch per core = 8
CB = 4                     # col blocks per batch (each 2048 preds)
CBW = N // CB              # col block width = 2048
NSTRIP = B * CB * RT       # 64 strips per core
K = 24                     # contraction rows of the augmented matmul

# engine assignment knobs (see module docstring)
DVE_EVICT_T = {0: True, 4: None}   # t=0 always DVE-evict; t=4 on even groups
GP_GROUPS = ()                     # (b*CB+cb) groups whose colacc runs on GPSIMD
                                   # (walrus rejects Pool TensorTensor on fp16)

_BF16 = ml_dtypes.bfloat16


def _ensure_concourse():
    for p in ("/root/.axon_site", "/root/.axon_site/_ro/trn_rl_repo",
              "/root/.axon_site/_ro/pypackages", "/opt/trn_rl_repo"):
        if os.path.isdir(p) and p not in sys.path:
            sys.path.append(p)


def _split3(x64):
    """Split a float64 array into three bf16 components summing to ~24 bits."""
    h = x64.astype(_BF16)
    r = x64 - h.astype(np.float64)
    m = r.astype(_BF16)
    r2 = r - m.astype(np.float64)
    l = r2.astype(_BF16)
    return h, m, l


def _build_aug(pred, gt):
    """Build aug_pred [K, B*N] and aug_gt [K, B*N] bf16 host arrays.

    Row pairing k: lhsT[k] (gt side) x rhs[k] (pred side):
      0-2   gh . Ph      3-5   gh . Pm      6-8   gm . Ph
      9-11  gh . Pl     12-14  gl . Ph     15-17  gm . Pm
      18-20 gsq{h,m,l} . (-1)              21-23  1 . (-psq{h,m,l})
    where P = 2*pred.
    """
    g64 = gt.astype(np.float64).reshape(B * N, 3)
    P64 = (2.0 * pred.astype(np.float64)).reshape(B * N, 3)
    gsq = (gt.astype(np.float32) ** 2).sum(-1, dtype=np.float32).astype(np.float64).reshape(B * N)
    psq = (pred.astype(np.float32) ** 2).sum(-1, dtype=np.float32).astype(np.float64).reshape(B * N)

    gh, gm, gl = _split3(g64)
    Ph, Pm, Pl = _split3(P64)
    gsqh, gsqm, gsql = _split3(gsq)
    psqh, psqm, psql = _split3(psq)

    one = np.ones(B * N, _BF16)
    neg1 = np.full(B * N, -1.0, _BF16)

    def rows3(a):  # [B*N, 3] -> 3 rows
        return [a[:, 0], a[:, 1], a[:, 2]]

    aug_gt = np.stack(
        rows3(gh) + rows3(gh) + rows3(gm) + rows3(gh) + rows3(gl) + rows3(gm)
        + [gsqh, gsqm, gsql, one, one, one], axis=0)
    aug_pred = np.stack(
        rows3(Ph) + rows3(Pm) + rows3(Ph) + rows3(Pl) + rows3(Ph) + rows3(Pm)
        + [neg1, neg1, neg1, -psqh, -psqm, -psql], axis=0)
    assert aug_gt.shape == (K, B * N) and aug_pred.shape == (K, B * N)
    return aug_gt, aug_pred


def _use_dve_evict(group, t):
    v = DVE_EVICT_T.get(t)
    if v is None:
        return group % 2 == 0
    return bool(v)


def build_nc():
    """Trace + compile the single-program SPMD kernel. Returns the Bacc."""
    _ensure_concourse()
    from contextlib import ExitStack
    import concourse.tile as tile
    from concourse import bacc, mybir

    f32 = mybir.dt.float32
    bf16 = mybir.dt.bfloat16
    f16 = mybir.dt.float16
    MAX = mybir.AluOpType.max
    ADD = mybir.AluOpType.add

    nc = bacc.Bacc("TRN2", target_bir_lowering=False, debug=False,
                   enable_asserts=False, num_devices=NCORES)
    ag_d = nc.dram_tensor("aug_gt", [K, B * GPC], bf16, kind="ExternalInput").ap()
    ap_d = nc.dram_tensor("aug_pred", [K, B * N], bf16, kind="ExternalInput").ap()
    rmax_d = nc.dram_tensor("rowmax_out", [128, B * RT], f32, kind="ExternalOutput").ap()
    cmax_d = nc.dram_tensor("colmax_out", [128, B * N], f16, kind="ExternalOutput").ap()

    with tile.TileContext(nc) as tc, ExitStack() as ctx:
        const_pool = ctx.enter_context(tc.tile_pool(name="const", bufs=1))
        psum_pool = ctx.enter_context(tc.tile_pool(name="ps", bufs=2, space="PSUM"))
        bpool = ctx.enter_context(tc.tile_pool(name="bs", bufs=6))
        fpool = ctx.enter_context(tc.tile_pool(name="fold", bufs=3))

        ag = const_pool.tile([K, B * GPC], bf16)
        nc.sync.dma_start(ag[:], ag_d[:])
        apt = const_pool.tile([K, B * N], bf16)
        nc.sync.dma_start(apt[:], ap_d[:])
        colacc = const_pool.tile([128, B * N], f16)
        rfin = const_pool.tile([128, B * RT], f32)

        # loop: row tile (b, t) outer, col block (cb) inner — a row tile's 4
        # strips are consecutive so its rowmax fold tree is local; the 4
        # colacc chains (per cb) have deps 4 strips apart (no stalls).
        for b in range(B):
            for t in range(RT):
                wcol = (b * RT + t) * 128
                folds = []
                strips = []
                for cb in range(CB):
                    ccol = b * N + cb * CBW
                    psum = psum_pool.tile([128, CBW], f32, tag="ps")
                    for n in range(4):
                        nc.tensor.matmul(
                            psum[:, n * 512:(n + 1) * 512],
                            lhsT=ag[:, wcol:wcol + 128],
                            rhs=apt[:, ccol + n * 512: ccol + (n + 1) * 512],
                            start=True, stop=True)
                    bstrip = bpool.tile([128, CBW], f16, tag="bs")
                    nc.scalar.activation(bstrip[:], psum[:],
                                         mybir.ActivationFunctionType.Copy)
                    strips.append(bstrip)
                    # colacc: chain per (b, cb) across t
                    cc = colacc[:, ccol:ccol + CBW]
                    if t == 0:
                        nc.vector.tensor_copy(cc[:], bstrip[:])
                    else:
                        nc.vector.tensor_tensor(out=cc[:], in0=cc[:],
                                                in1=bstrip[:], op=MAX)
                    # rowmax fold tree (2x fp16 tt pair folds)
                    if cb % 2 == 1:
                        f = fpool.tile([128, CBW], f16, tag="f")
                        nc.vector.tensor_tensor(out=f[:], in0=strips[cb - 1][:],
                                                in1=bstrip[:], op=MAX)
                        folds.append(f)
                # final: fold 2048 -> 1024 -> 512 at 2x, then 1x reduce on 512
                # (tensor_tensor_reduce would fuse this but crashes TRN2 HW)
                rcol = b * RT + t
                f = fpool.tile([128, CBW], f16, tag="f2")
                nc.vector.tensor_tensor(out=f[:], in0=folds[0][:],
                                        in1=folds[1][:], op=MAX)
                g = fpool.tile([128, CBW // 2], f16, tag="g")
                nc.vector.tensor_tensor(out=g[:], in0=f[:, 0:CBW // 2],
                                        in1=f[:, CBW // 2:CBW], op=MAX)
                h = fpool.tile([128, CBW // 4], f16, tag="h")
                nc.vector.tensor_tensor(out=h[:], in0=g[:, 0:CBW // 4],
                                        in1=g[:, CBW // 4:CBW // 2], op=MAX)
                # NOTE: tensor_reduce here (interleaved with the in-place
                # colacc tensor_tensors) hangs TRN2 hardware; the tensor_scalar
                # accumulate path (CACHE_REDUCE) is stable.
                hd = fpool.tile([128, CBW // 4], f16, tag="hd")
                nc.vector.tensor_scalar(
                    out=hd[:], in0=h[:], scalar1=0.0, scalar2=None,
                    op0=ADD, op1=MAX, accum_out=rfin[:, rcol:rcol + 1])

            # batch b's colacc chunks are final here — overlap DMA-out with
            # the next batch's compute
            for cb in range(CB):
                ccol = b * N + cb * CBW
                nc.sync.dma_start(cmax_d[:, ccol:ccol + CBW],
                                  colacc[:, ccol:ccol + CBW])
        nc.sync.dma_start(rmax_d[:], rfin[:])

    nc.compile()
    return nc


_NC_CACHE = None


def _get_nc():
    global _NC_CACHE
    if _NC_CACHE is None:
        _NC_CACHE = build_nc()
    return _NC_CACHE


def make_in_maps(pred, gt):
    """Per-core input dicts. Core c gets gt rows [c*GPC, (c+1)*GPC) of each
    batch (aug_gt columns laid out b-major: (b*RT + t)*128 + p)."""
    aug_gt, aug_pred = _build_aug(pred, gt)
    ag_bn = aug_gt.reshape(K, B, N)
    in_maps = []
    for c in range(NCORES):
        ag_c = ag_bn[:, :, c * GPC:(c + 1) * GPC].reshape(K, B * GPC)
        in_maps.append({"aug_gt": np.ascontiguousarray(ag_c),
                        "aug_pred": np.ascontiguousarray(aug_pred)})
    return in_maps


def finalize(results):
    """Host finale: negated maxes -> mins -> sqrt -> means."""
    # rowmax_out: [128, B*RT], col = b*RT + t, partition p -> gt row c*GPC + t*128 + p
    dist1_sq = np.empty((B, N), np.float64)
    for c in range(NCORES):
        r = np.asarray(results[c]["rowmax_out"], np.float64)  # [128, B*RT]
        r = r.reshape(128, B, RT).transpose(1, 2, 0).reshape(B, GPC)
        dist1_sq[:, c * GPC:(c + 1) * GPC] = -r
    # colmax_out: [128, B*N] fp16 per core; fold cores and partitions
    call = np.stack([np.asarray(results[c]["colmax_out"]).astype(np.float32)
                     for c in range(NCORES)], axis=0)  # [8, 128, B*N]
    dist2_sq = -(call.max(axis=(0, 1)).astype(np.float64).reshape(B, N))

    dist1 = np.sqrt(np.maximum(dist1_sq, 0.0))
    dist2 = np.sqrt(np.maximum(dist2_sq, 0.0))
    chamfer = (dist1.mean(axis=1) + dist2.mean(axis=1)).mean()
    return np.float32(chamfer)


def kernel(pred, gt):
    _ensure_concourse()
    pred = np.asarray(pred, dtype=np.float32)
    gt = np.asarray(gt, dtype=np.float32)
    assert pred.shape == (B, N, 3) and gt.shape == (B, N, 3)

    in_maps = make_in_maps(pred, gt)
    nc = _get_nc()
    from concourse import bass_utils
    res = bass_utils.run_bass_kernel_spmd(nc, in_maps, core_ids=list(range(NCORES)))
    return finalize(res.results)
